# revision 1
# baseline (speedup 1.0000x reference)
"""CrAKNConvV2 GNN message-passing kernel for 8 trn2 NeuronCores (Bass/Tile).

Self-contained; shapes hardcoded for N=40000, E=200000, D=128, K=2.

Strategy:
  - Edges sharded across 8 cores by dst octile (5000 nodes/core), sorted by
    dst and packed into 512-edge macro-tiles whose dst window spans <= 128
    local nodes.  Segment-sum becomes a one-hot [e,u] matmul per macro plus a
    128-row indirect scatter into a per-core h table (no all-reduce needed).
  - Node stage (q/k/v) computed per-octile; q AllGathered (bf16) so each core
    can gather q[src] for arbitrary global src.
  - Edge MLPs in "D-on-partition" layout: tiles [do, e] with weights as
    stationary lhsT; LN mean folded into W1 on the host; variance via a
    replicated ones-matmul; rsqrt on ACT (single abs_reciprocal_sqrt table
    set); mish via a (5,4) rational with custom DVE ops + fast reciprocal.
  - Comb MLP on the local h slice; output returned transposed per core and
    assembled on the host.
"""
import sys

sys.path.insert(0, "/opt/trn_rl_repo")

import numpy as np
import ml_dtypes

import concourse.bass as bass
import concourse.bacc as bacc
import concourse.tile as tile
import concourse.mybir as mybir
from concourse.bass_utils import run_bass_kernel_spmd
from concourse.masks import make_identity

bf16 = ml_dtypes.bfloat16
dt = mybir.dt
AF = mybir.ActivationFunctionType
ALU = mybir.AluOpType

N, E, D, K = 40000, 200000, 128, 2
NC = 8
NPC = N // NC            # 5000 nodes per core
V = 5120                 # padded local node-table rows (real: 0..4999)
SUB = 128
MAC = 512                # edges per macro tile (4 subtiles)
NCM = V // MAC           # node/comb macros per core (10)

# (5,4) rational fit of mish on [-7, 7], max abs err 2.2e-3:
#   mish(x) ~= S * x*((((x+a)x+b)x+c)x+d) / ((((x+q3)x+q2)x+q1)x+q0)
MISH_A = 19.166871033044686
MISH_B = 153.65177897609692
MISH_C = 677.6408201640626
MISH_D = 1431.337455343249
MISH_Q3 = -5.812366573366565
MISH_Q2 = 144.26844999380356
MISH_Q1 = -42.68836425004621
MISH_Q0 = 836.7746892765476
MISH_S = 0.35327729144764103

LN_EPS = 1e-5
RAD_EPS = 1e-8

_MISH_OPS = {}


def _register_mish_ops():
    """Register MISH_DEN / MISH_NUM custom DVE ops (monic quartic Horner with
    the constant term spilled to in1)."""
    if _MISH_OPS:
        return _MISH_OPS
    import concourse.dve_ops as dve_ops
    from concourse.dve_spec import Spec, Src0, Src1, C0, C1, C2, lower
    from concourse.dve_uop import DveOpSpec

    def _ref_den(in0, in1, s0, s1, imm2):
        x = in0.astype(np.float32)
        return ((((x + s0) * x + s1) * x + imm2) * x) + in1

    def _ref_num(in0, in1, s0, s1, imm2):
        x = in0.astype(np.float32)
        return (((((x + s0) * x + s1) * x + imm2) * x) + in1) * x

    body_den = ((((Src0 + C0) * Src0 + C1) * Src0 + C2) * Src0) + Src1
    body_num = (((((Src0 + C0) * Src0 + C1) * Src0 + C2) * Src0) + Src1) * Src0

    made = []
    for name, body, ref in (
        ("MISH_DEN_ANT", body_den, _ref_den),
        ("MISH_NUM_ANT", body_num, _ref_num),
    ):
        existing = [o for o in dve_ops.OPS if o.name == name]
        if existing:
            made.append(existing[0])
            continue
        row = max(dve_ops._SUB_OPCODE_FOR_NAME.values()) + 1
        assert row < 0x20
        dve_ops._SUB_OPCODE_FOR_NAME[name] = row
        spec = Spec(body=body, reference=ref)
        shas = {}
        for ver in ("v3", "v4"):
            try:
                shas[ver] = DveOpSpec(name=name, opcode=row,
                                      uops=lower(spec, ver=ver),
                                      rd1_en=True).sha(ver)
            except Exception:
                pass
        op = dve_ops.DveOp(name, spec, subdim=False, uops_sha=shas)
        dve_ops.OPS.append(op)
        dve_ops.CUSTOM_DVE_SPECS[name] = spec
        made.append(op)
    _MISH_OPS["den"] = made[0]
    _MISH_OPS["num"] = made[1]
    return _MISH_OPS


# ---------------------------------------------------------------- host prep
def _fold_ln_w(W, b):
    return W - W.mean(axis=1, keepdims=True), b - b.mean()


def _pack_core(deg):
    macros = []
    n = 0
    e0 = 0
    while n < NPC:
        first = n
        edges = 0
        while n < NPC and (n - first) < SUB and edges + deg[n] <= MAC:
            edges += int(deg[n])
            n += 1
        assert n > first, f"node {n} has degree {deg[n]} > {MAC}"
        macros.append((first, n - first, e0, edges))
        e0 += edges
    return macros


def _prep(inputs):
    nf = np.ascontiguousarray(np.asarray(inputs["node_feat"], np.float32))
    src = np.asarray(inputs["src"], np.int32)
    dst = np.asarray(inputs["dst"], np.int32)
    ef = np.asarray(inputs["edge_feat"], np.float32)

    Wq, bq = _fold_ln_w(np.asarray(inputs["Wq"], np.float32), np.asarray(inputs["bq"], np.float32))
    Wk, bk = _fold_ln_w(np.asarray(inputs["Wk"], np.float32), np.asarray(inputs["bk"], np.float32))
    Wv = np.asarray(inputs["Wv"], np.float32)
    bv = np.asarray(inputs["bv"], np.float32)
    gq, beq = np.asarray(inputs["gq"], np.float32), np.asarray(inputs["beta_q"], np.float32)
    gk, bek = np.asarray(inputs["gk"], np.float32), np.asarray(inputs["beta_k"], np.float32)

    mW1 = np.asarray(inputs["mlp_W1"], np.float32).copy()   # [3,K,D,D]
    mb1 = np.asarray(inputs["mlp_b1"], np.float32).copy()   # [3,K,D]
    mg = np.asarray(inputs["mlp_g"], np.float32)
    mbe = np.asarray(inputs["mlp_beta"], np.float32)
    mW2 = np.asarray(inputs["mlp_W2"], np.float32)
    mb2 = np.asarray(inputs["mlp_b2"], np.float32)
    for i in range(3):
        for k in range(K):
            mW1[i, k], mb1[i, k] = _fold_ln_w(mW1[i, k], mb1[i, k])

    cW1 = np.asarray(inputs["comb_W1"], np.float32)
    cb1 = np.asarray(inputs["comb_b1"], np.float32)
    cW2 = np.asarray(inputs["comb_W2"], np.float32)

    gen_node = not (np.all(gq == 1) and np.all(beq == 0)
                    and np.all(gk == 1) and np.all(bek == 0))
    gen_edge = not (np.all(mg == 1) and np.all(mbe == 0))

    order = np.argsort(dst, kind="stable")
    dsts = dst[order]
    core_bounds = np.searchsorted(dsts, np.arange(NC + 1) * NPC)

    per_core = []
    NMs = []
    for c in range(NC):
        eidx = order[core_bounds[c]:core_bounds[c + 1]]
        dl = dsts[core_bounds[c]:core_bounds[c + 1]] - c * NPC
        deg = np.bincount(dl, minlength=NPC)
        macros = _pack_core(deg)
        per_core.append((eidx, dl, macros))
        NMs.append(len(macros))
    NM = max(NMs)

    cores = []
    for c in range(NC):
        eidx, dl, macros = per_core[c]
        nmc = len(macros)
        efT = np.zeros((K, D, NM * MAC), bf16)
        srcq = np.zeros((NM, SUB, 4), np.int32)
        win = np.zeros((NM, SUB, 1), np.int32)
        scat = np.zeros((NM, SUB, 1), np.int32)
        pool = np.zeros((NM, SUB, 4, SUB), bf16)
        poolT = np.zeros((NM, SUB, 4, SUB), bf16)
        ar = np.arange(SUB)
        for m in range(NM):
            if m < nmc:
                first, span, e0, ne = macros[m]
            else:
                first, span, e0, ne = NPC, 0, len(eidx), 0
            win[m, :, 0] = np.minimum(first + ar, V - 1)
            scat[m, :, 0] = np.where(ar < span, first + ar,
                                     NPC + ((ar - span) % (V - NPC)))
            if ne == 0:
                continue
            ge = eidx[e0:e0 + ne]
            doff = dl[e0:e0 + ne] - first
            slots = np.arange(ne)
            cc, pp = slots // SUB, slots % SUB
            cols = m * MAC + slots
            for k in range(K):
                efT[k][:, cols] = ef[k][ge].T.astype(bf16)
            s_glob = src[ge]
            srcq[m][pp, cc] = (s_glob // NPC) * V + (s_glob % NPC)
            pool[m][pp, cc, doff] = 1
            poolT[m][doff, cc, pp] = 1
        nfT = np.zeros((D, V), bf16)
        nfT[:, :NPC] = nf[c * NPC:(c + 1) * NPC].T.astype(bf16)
        cores.append(dict(
            efT=efT, srcq=srcq, win=win, scat=scat,
            pool=pool.reshape(NM, SUB, 4 * SUB),
            poolT=poolT.reshape(NM, SUB, 4 * SUB), nfT=nfT))

    shared = dict(
        wq=Wq.astype(bf16), wk=Wk.astype(bf16), wv=Wv.astype(bf16),
        nbias=np.stack([bq, bk, bv])[None, :, :].astype(bf16),     # [1,3,D]
        ngam=np.stack([gq, gk])[None, :, :].astype(bf16),          # [1,2,D]
        nbet=np.stack([beq, bek])[None, :, :].astype(bf16),        # [1,2,D]
        mW1=mW1.transpose(2, 0, 1, 3).reshape(D, 3 * K * D).astype(bf16).copy(),
        mW2=mW2.transpose(2, 0, 1, 3).reshape(D, 3 * K * D).astype(bf16).copy(),
        mscal=np.stack([mb1, mb2, mg, mbe], axis=0)                # [4,3,K,D]
            .transpose(3, 0, 1, 2).reshape(D, 4 * 3 * K)
            .astype(np.float32).copy(),
        cW1=cW1.reshape(2, D, 2, D).transpose(1, 0, 2, 3)
            .reshape(D, 4 * D).astype(bf16).copy(),                # [di, (kc,mc,do)]
        cW2=cW2.reshape(2, D, D).transpose(1, 0, 2)
            .reshape(D, 2 * D).astype(bf16).copy(),                # [di, (mc,do)]
        cb1=cb1.reshape(2, D).T.astype(np.float32).copy(),         # [D, 2]
    )
    return cores, shared, NM, gen_node, gen_edge


# ---------------------------------------------------------------- builder
def _build(NM, gen_node, gen_edge):
    ops = _register_mish_ops()
    from concourse.dve_ops import RECIPROCAL_APPROX_FAST, RECIP_APPROX_FAST_CONSTS
    RC = RECIP_APPROX_FAST_CONSTS

    nc = bacc.Bacc("TRN2", target_bir_lowering=False, debug=False,
                   enable_asserts=False, num_devices=NC)

    t_nfT = nc.dram_tensor("nfT", [D, V], dt.bfloat16, kind="ExternalInput")
    t_wq = nc.dram_tensor("wq", [D, D], dt.bfloat16, kind="ExternalInput")
    t_wk = nc.dram_tensor("wk", [D, D], dt.bfloat16, kind="ExternalInput")
    t_wv = nc.dram_tensor("wv", [D, D], dt.bfloat16, kind="ExternalInput")
    t_nbias = nc.dram_tensor("nbias", [1, 3, D], dt.bfloat16, kind="ExternalInput")
    t_ngam = nc.dram_tensor("ngam", [1, 2, D], dt.bfloat16, kind="ExternalInput")
    t_nbet = nc.dram_tensor("nbet", [1, 2, D], dt.bfloat16, kind="ExternalInput")
    t_efT = nc.dram_tensor("efT", [K, D, NM * MAC], dt.bfloat16, kind="ExternalInput")
    t_srcq = nc.dram_tensor("srcq", [NM, SUB, 4], dt.int32, kind="ExternalInput")
    t_win = nc.dram_tensor("win", [NM, SUB, 1], dt.int32, kind="ExternalInput")
    t_scat = nc.dram_tensor("scat", [NM, SUB, 1], dt.int32, kind="ExternalInput")
    t_pool = nc.dram_tensor("pool", [NM, SUB, 4 * SUB], dt.bfloat16, kind="ExternalInput")
    t_poolT = nc.dram_tensor("poolT", [NM, SUB, 4 * SUB], dt.bfloat16, kind="ExternalInput")
    t_mW1 = nc.dram_tensor("mW1", [D, 3 * K * D], dt.bfloat16, kind="ExternalInput")
    t_mW2 = nc.dram_tensor("mW2", [D, 3 * K * D], dt.bfloat16, kind="ExternalInput")
    t_mscal = nc.dram_tensor("mscal", [D, 4 * 3 * K], dt.float32, kind="ExternalInput")
    t_cW1 = nc.dram_tensor("cW1", [D, 4 * D], dt.bfloat16, kind="ExternalInput")
    t_cW2 = nc.dram_tensor("cW2", [D, 2 * D], dt.bfloat16, kind="ExternalInput")
    t_cb1 = nc.dram_tensor("cb1", [D, 2], dt.float32, kind="ExternalInput")
    t_out = nc.dram_tensor("outT", [D, V], dt.float32, kind="ExternalOutput")

    with tile.TileContext(nc) as tc:
        with (
            tc.tile_pool(name="const", bufs=1) as cp,
            tc.tile_pool(name="sb", bufs=3) as sb,
            tc.tile_pool(name="mlp", bufs=2) as mp,
            tc.tile_pool(name="psA", bufs=3, space="PSUM") as psA,
            tc.tile_pool(name="psB", bufs=2, space="PSUM") as psB,
            tc.tile_pool(name="psC", bufs=1, space="PSUM") as psC,
            tc.tile_pool(name="dram", bufs=1, space="DRAM") as dram,
        ):
            # ---------------- constants
            ones = cp.tile([D, D], dt.bfloat16)
            nc.vector.memset(ones[:], 1.0)
            onesrow = cp.tile([1, D], dt.bfloat16)
            nc.vector.memset(onesrow[:], 1.0)
            ident = cp.tile([D, D], dt.bfloat16)
            make_identity(nc, ident[:])
            eps_ln = cp.tile([D, 1], dt.float32)
            nc.vector.memset(eps_ln[:], LN_EPS)
            eps_rad = cp.tile([D, 1], dt.float32)
            nc.vector.memset(eps_rad[:], RAD_EPS)
            mishd = cp.tile([D, 1], dt.float32)
            nc.vector.memset(mishd[:], MISH_D)
            mishq0 = cp.tile([D, 1], dt.float32)
            nc.vector.memset(mishq0[:], MISH_Q0)

            wq_t = cp.tile([D, D], dt.bfloat16); nc.sync.dma_start(wq_t[:], t_wq[:])
            wk_t = cp.tile([D, D], dt.bfloat16); nc.sync.dma_start(wk_t[:], t_wk[:])
            wv_t = cp.tile([D, D], dt.bfloat16); nc.sync.dma_start(wv_t[:], t_wv[:])
            nbias_t = cp.tile([1, 3, D], dt.bfloat16); nc.sync.dma_start(nbias_t[:], t_nbias[:])
            ngam_t = cp.tile([1, 2, D], dt.bfloat16); nc.sync.dma_start(ngam_t[:], t_ngam[:])
            nbet_t = cp.tile([1, 2, D], dt.bfloat16); nc.sync.dma_start(nbet_t[:], t_nbet[:])
            mW1_t = cp.tile([D, 3 * K * D], dt.bfloat16); nc.sync.dma_start(mW1_t[:], t_mW1[:])
            mW2_t = cp.tile([D, 3 * K * D], dt.bfloat16); nc.sync.dma_start(mW2_t[:], t_mW2[:])
            mscal_t = cp.tile([D, 4 * 3 * K], dt.float32); nc.sync.dma_start(mscal_t[:], t_mscal[:])
            cW1_t = cp.tile([D, 4 * D], dt.bfloat16); nc.sync.dma_start(cW1_t[:], t_cW1[:])
            cW2_t = cp.tile([D, 2 * D], dt.bfloat16); nc.sync.dma_start(cW2_t[:], t_cW2[:])
            cb1_t = cp.tile([D, 2], dt.float32); nc.sync.dma_start(cb1_t[:], t_cb1[:])

            def msc(which, i, k):
                col = which * 6 + i * K + k
                return mscal_t[:, col:col + 1]

            def w1s(i, k):
                b = (i * K + k) * D
                return mW1_t[:, b:b + D]

            def w2s(i, k):
                b = (i * K + k) * D
                return mW2_t[:, b:b + D]

            def cw1s(kc, mc):
                b = (kc * 2 + mc) * D
                return cW1_t[:, b:b + D]

            if gen_node:
                ngb = []
                for j in range(2):
                    gp = psB.tile([D, D], dt.float32, tag="small", space="PSUM")
                    nc.tensor.matmul(gp[:], onesrow[:], ngam_t[:, j, :], start=True, stop=True)
                    gs = cp.tile([D, D], dt.bfloat16, tag=f"ngb{j}")
                    nc.vector.tensor_copy(gs[:], gp[:])
                    bp = psB.tile([D, D], dt.float32, tag="small", space="PSUM")
                    nc.tensor.matmul(bp[:], onesrow[:], nbet_t[:, j, :], start=True, stop=True)
                    bs = cp.tile([D, D], dt.bfloat16, tag=f"nbb{j}")
                    nc.vector.tensor_copy(bs[:], bp[:])
                    ngb.append((gs, bs))

            q_local = dram.tile([V, D], dt.bfloat16)
            q_all = dram.tile([NC * V, D], dt.bfloat16, addr_space="Shared")
            kv_local = dram.tile([V, 2 * D], dt.bfloat16)
            h_local = dram.tile([V, 2 * D], dt.bfloat16)

            # ---------------- mish helper
            def mish_apply(x_ap, P, F):
                qd = mp.tile([P, F], dt.float32, tag="mish_q")
                nc.vector._custom_dve(ops["den"], out=qd[:], in0=x_ap,
                                      in1=mishq0[:P].to_broadcast([P, F]),
                                      s0=MISH_Q3, s1=MISH_Q2, imm2=MISH_Q1)
                rr = mp.tile([P, F], dt.float32, tag="mish_r")
                nc.vector._custom_dve(RECIPROCAL_APPROX_FAST, out=rr[:], in0=qd[:],
                                      s0=RC["s0"], s1=RC["s1"], imm2=RC["imm2"])
                pn = mp.tile([P, F], dt.float32, tag="mish_p")
                nc.vector._custom_dve(ops["num"], out=pn[:], in0=x_ap,
                                      in1=mishd[:P].to_broadcast([P, F]),
                                      s0=MISH_A, s1=MISH_B, imm2=MISH_C)
                z = mp.tile([P, F], dt.bfloat16, tag="mish_z")
                nc.vector.scalar_tensor_tensor(z[:], pn[:], MISH_S, rr[:],
                                               op0=ALU.mult, op1=ALU.mult)
                return z

            # ================ node stage ================
            for nm in range(NCM):
                n0 = nm * MAC
                nfT_sl = sb.tile([D, MAC], dt.bfloat16, tag="nf")
                nc.sync.dma_start(nfT_sl[:], t_nfT[:, n0:n0 + MAC])
                for chain in range(3):      # 0=q 1=k 2=v
                    W_t = (wq_t, wk_t, wv_t)[chain]
                    xm = sb.tile([SUB, 4, D], dt.bfloat16, tag="node_x")
                    if chain < 2:
                        ss4 = sb.tile([SUB, 4], dt.float32, tag="node_ss")
                        s4 = sb.tile([SUB, 4], dt.float32, tag="node_s4")
                        for c in range(4):
                            u = psB.tile([SUB, D], dt.float32, tag="small",
                                         space="PSUM")
                            nc.tensor.matmul(u[:], nfT_sl[:, c * SUB:(c + 1) * SUB],
                                             W_t[:], start=True, stop=False)
                            nc.tensor.matmul(u[:], onesrow[:], nbias_t[:, chain, :],
                                             start=False, stop=True)
                            y2 = sb.tile([SUB, D], dt.bfloat16, tag="node_y2")
                            nc.scalar.activation(y2[:], u[:], AF.Square,
                                                 accum_out=ss4[:, c:c + 1])
                            nc.scalar.activation(s4[:, c:c + 1], ss4[:, c:c + 1],
                                                 AF.Abs_reciprocal_sqrt,
                                                 bias=eps_ln[:], scale=1.0 / D)
                            nc.vector.tensor_scalar(xm[:, c, :], u[:],
                                                    s4[:, c:c + 1], None,
                                                    op0=ALU.mult)
                        if gen_node:
                            gs, bs = ngb[chain]
                            xg = sb.tile([SUB, 4, D], dt.bfloat16, tag="node_xg")
                            for c in range(4):
                                nc.vector.tensor_tensor(xg[:, c, :], xm[:, c, :],
                                                        gs[:], op=ALU.mult)
                                nc.vector.tensor_tensor(xm[:, c, :], xg[:, c, :],
                                                        bs[:], op=ALU.add)
                        zm = mish_apply(xm[:].rearrange("p c d -> p (c d)"), SUB, 4 * D)
                        rows = zm[:].rearrange("p (c d) -> p c d", c=4)
                    else:
                        for c in range(4):
                            u = psB.tile([SUB, D], dt.float32, tag="small",
                                         space="PSUM")
                            nc.tensor.matmul(u[:], nfT_sl[:, c * SUB:(c + 1) * SUB],
                                             W_t[:], start=True, stop=False)
                            nc.tensor.matmul(u[:], onesrow[:], nbias_t[:, chain, :],
                                             start=False, stop=True)
                            nc.vector.tensor_copy(xm[:, c, :], u[:])
                        rows = xm[:]
                    if chain == 0:
                        dst_ap = q_local[n0:n0 + MAC, :].rearrange(
                            "(c p) d -> p c d", p=SUB)
                    elif chain == 1:
                        dst_ap = kv_local[n0:n0 + MAC, 0:D].rearrange(
                            "(c p) d -> p c d", p=SUB)
                    else:
                        dst_ap = kv_local[n0:n0 + MAC, D:2 * D].rearrange(
                            "(c p) d -> p c d", p=SUB)
                    nc.sync.dma_start(dst_ap, rows)

            # ================ allgather q ================
            nc.gpsimd.collective_compute(
                "AllGather", ALU.bypass,
                replica_groups=[list(range(NC))],
                ins=[q_local.opt()], outs=[q_all.opt()])

            # ================ edge stage ================
            for m in range(NM):
                sl = slice(m * MAC, (m + 1) * MAC)
                efk = []
                for k in range(K):
                    t = sb.tile([D, MAC], dt.bfloat16, tag=f"ef{k}")
                    nc.sync.dma_start(t[:], t_efT[k][:, sl])
                    efk.append(t)
                pool_sb = sb.tile([SUB, 4 * SUB], dt.bfloat16, tag="pool")
                nc.sync.dma_start(pool_sb[:], t_pool[m])
                poolT_sb = sb.tile([SUB, 4 * SUB], dt.bfloat16, tag="poolT")
                nc.sync.dma_start(poolT_sb[:], t_poolT[m])
                srcq_sb = sb.tile([SUB, 4], dt.int32, tag="srcq")
                nc.sync.dma_start(srcq_sb[:], t_srcq[m])
                win_sb = sb.tile([SUB, 1], dt.int32, tag="win")
                nc.sync.dma_start(win_sb[:], t_win[m])
                scat_sb = sb.tile([SUB, 1], dt.int32, tag="scat")
                nc.sync.dma_start(scat_sb[:], t_scat[m])

                qg = sb.tile([SUB, 4, D], dt.bfloat16, tag="qg")
                for c in range(4):
                    nc.gpsimd.indirect_dma_start(
                        out=qg[:, c, :], out_offset=None, in_=q_all[:],
                        in_offset=bass.IndirectOffsetOnAxis(
                            ap=srcq_sb[:, c:c + 1], axis=0))
                kvw = sb.tile([SUB, 2 * D], dt.bfloat16, tag="kvw")
                nc.gpsimd.indirect_dma_start(
                    out=kvw[:], out_offset=None, in_=kv_local[:],
                    in_offset=bass.IndirectOffsetOnAxis(ap=win_sb[:], axis=0))

                kve = sb.tile([SUB, 4, 2 * D], dt.bfloat16, tag="kve")
                for c in range(4):
                    kp = psB.tile([SUB, 2 * D], dt.float32, tag="small", space="PSUM")
                    nc.tensor.matmul(kp[:], poolT_sb[:, c * SUB:(c + 1) * SUB],
                                     kvw[:], start=True, stop=True)
                    nc.vector.tensor_copy(kve[:, c, :], kp[:])

                rel = sb.tile([SUB, 4, D], dt.bfloat16, tag="rel")
                nc.vector.tensor_tensor(rel[:], qg[:], kve[:, :, 0:D],
                                        op=ALU.subtract)
                rel2 = sb.tile([SUB, 4, D], dt.bfloat16, tag="rel2")
                nc.vector.tensor_tensor(rel2[:], rel[:], rel[:], op=ALU.mult)
                rad = sb.tile([SUB, 4], dt.float32, tag="rad")
                nc.vector.tensor_reduce(rad[:], rel2[:], axis=mybir.AxisListType.X,
                                        op=ALU.add)
                invr = sb.tile([SUB, 4], dt.float32, tag="invr")
                nc.scalar.activation(invr[:], rad[:], AF.Abs_reciprocal_sqrt,
                                     bias=eps_rad[:], scale=1.0)
                reln = sb.tile([SUB, 4, D], dt.bfloat16, tag="reln")
                for c in range(4):
                    nc.vector.tensor_scalar(reln[:, c, :], rel[:, c, :],
                                            invr[:, c:c + 1], None, op0=ALU.mult)
                relT = sb.tile([D, MAC], dt.bfloat16, tag="relT")
                for c in range(4):
                    rp = psB.tile([SUB, SUB], dt.bfloat16, tag="small", space="PSUM")
                    nc.tensor.transpose(rp[:], reln[:, c, :], ident[:])
                    nc.vector.tensor_copy(relT[:, c * SUB:(c + 1) * SUB], rp[:])

                H = psC.tile([SUB, 2 * D], dt.float32, tag="H", space="PSUM")

                def edge_ln_mish(u_ps, i, k):
                    """u_ps: [D, MAC] PSUM (pre-bias).  Returns mish(LN(u+b1))."""
                    y = mp.tile([D, MAC], dt.bfloat16, tag="ey")
                    nc.vector.tensor_scalar(y[:], u_ps[:], msc(0, i, k), None,
                                            op0=ALU.add)
                    y2 = mp.tile([D, MAC], dt.bfloat16, tag="ey2")
                    nc.scalar.activation(y2[:], u_ps[:], AF.Square,
                                         bias=msc(0, i, k))
                    mt = psA.tile([D, MAC], dt.float32, tag="big", space="PSUM")
                    nc.tensor.matmul(mt[:], ones[:], y2[:], start=True, stop=True)
                    s = mp.tile([D, MAC], dt.bfloat16, tag="es")
                    nc.scalar.activation(s[:], mt[:], AF.Abs_reciprocal_sqrt,
                                         bias=eps_ln[:], scale=1.0 / D)
                    x = mp.tile([D, MAC], dt.bfloat16, tag="ex")
                    nc.vector.tensor_tensor(x[:], y[:], s[:], op=ALU.mult)
                    if gen_edge:
                        xg = mp.tile([D, MAC], dt.bfloat16, tag="exg")
                        nc.vector.tensor_scalar(xg[:], x[:], msc(2, i, k),
                                                msc(3, i, k), op0=ALU.mult,
                                                op1=ALU.add)
                        x = xg
                    return mish_apply(x[:], D, MAC)

                for k in range(K):
                    um = psA.tile([D, MAC], dt.float32, tag="big", space="PSUM")
                    nc.tensor.matmul(um[:], w1s(0, k), efk[k][:], start=True, stop=True)
                    zm = edge_ln_mish(um, 0, k)
                    ub = psA.tile([D, MAC], dt.float32, tag="big", space="PSUM")
                    nc.tensor.matmul(ub[:], w1s(1, k), efk[k][:], start=True, stop=True)
                    zb = edge_ln_mish(ub, 1, k)
                    pem = psB.tile([D, MAC], dt.float32, tag="eu2", space="PSUM")
                    nc.tensor.matmul(pem[:], w2s(0, k), zm[:], start=True, stop=True)
                    peb = psB.tile([D, MAC], dt.float32, tag="eu2", space="PSUM")
                    nc.tensor.matmul(peb[:], w2s(1, k), zb[:], start=True, stop=True)
                    rhalf = mp.tile([D, MAC], dt.bfloat16, tag="rh")
                    nc.vector.scalar_tensor_tensor(rhalf[:], pem[:], msc(1, 0, k),
                                                   relT[:], op0=ALU.add, op1=ALU.mult)
                    rcomb = mp.tile([D, MAC], dt.bfloat16, tag="rc")
                    nc.vector.scalar_tensor_tensor(rcomb[:], peb[:], msc(1, 1, k),
                                                   rhalf[:], op0=ALU.add, op1=ALU.add)
                    uw = psA.tile([D, MAC], dt.float32, tag="big", space="PSUM")
                    nc.tensor.matmul(uw[:], w1s(2, k), rcomb[:], start=True, stop=True)
                    zw = edge_ln_mish(uw, 2, k)
                    wu = psB.tile([D, MAC], dt.float32, tag="eu2", space="PSUM")
                    nc.tensor.matmul(wu[:], w2s(2, k), zw[:], start=True, stop=True)
                    wsb = mp.tile([D, MAC], dt.bfloat16, tag="ws")
                    nc.vector.tensor_scalar(wsb[:], wu[:], msc(1, 2, k), None,
                                            op0=ALU.add)
                    for c in range(4):
                        wp = psB.tile([SUB, SUB], dt.bfloat16, tag="small", space="PSUM")
                        nc.tensor.transpose(wp[:], wsb[:, c * SUB:(c + 1) * SUB],
                                            ident[:])
                        val = mp.tile([SUB, SUB], dt.bfloat16, tag="val")
                        nc.vector.tensor_tensor(val[:], wp[:], kve[:, c, D:2 * D],
                                                op=ALU.mult)
                        nc.tensor.matmul(H[:, k * D:(k + 1) * D],
                                         pool_sb[:, c * SUB:(c + 1) * SUB],
                                         val[:], start=(c == 0), stop=(c == 3))
                Hs = sb.tile([SUB, 2 * D], dt.bfloat16, tag="Hs")
                nc.vector.tensor_copy(Hs[:], H[:])
                nc.gpsimd.indirect_dma_start(
                    out=h_local[:], out_offset=bass.IndirectOffsetOnAxis(
                        ap=scat_sb[:], axis=0),
                    in_=Hs[:], in_offset=None)

            # ================ comb stage ================
            for cm in range(NCM):
                n0 = cm * MAC
                hT = []
                for j in range(2):
                    t = sb.tile([D, MAC], dt.bfloat16, tag=f"hT{j}")
                    nc.sync.dma_start_transpose(
                        t[:], h_local[n0:n0 + MAC, j * D:(j + 1) * D])
                    hT.append(t)
                zc = []
                for mc in range(2):
                    cu = psA.tile([D, MAC], dt.float32, tag="big", space="PSUM")
                    nc.tensor.matmul(cu[:], cw1s(0, mc), hT[0][:], start=True, stop=False)
                    nc.tensor.matmul(cu[:], cw1s(1, mc), hT[1][:], start=False, stop=True)
                    y = mp.tile([D, MAC], dt.bfloat16, tag="ey")
                    nc.vector.tensor_scalar(y[:], cu[:], cb1_t[:, mc:mc + 1], None, op0=ALU.add)
                    zc.append(mish_apply(y[:], D, MAC))
                ou = psB.tile([D, MAC], dt.float32, tag="eu2", space="PSUM")
                nc.tensor.matmul(ou[:], cW2_t[:, 0:D], zc[0][:], start=True, stop=False)
                nc.tensor.matmul(ou[:], cW2_t[:, D:2 * D], zc[1][:], start=False, stop=True)
                osb = sb.tile([D, MAC], dt.float32, tag="osb")
                nc.vector.tensor_copy(osb[:], ou[:])
                nc.sync.dma_start(t_out[:, n0:n0 + MAC], osb[:])

    nc.compile()
    return nc


_CACHE = {}


def kernel(**inputs) -> np.ndarray:
    cores, shared, NM, gen_node, gen_edge = _prep(inputs)
    key = (NM, gen_node, gen_edge)
    if key not in _CACHE:
        _CACHE[key] = _build(NM, gen_node, gen_edge)
    nc = _CACHE[key]
    in_maps = []
    for c in range(NC):
        m = dict(shared)
        m.update(cores[c])
        in_maps.append(m)
    res = run_bass_kernel_spmd(nc, in_maps, core_ids=list(range(NC)))
    out = np.empty((N, D), np.float32)
    for c in range(NC):
        out[c * NPC:(c + 1) * NPC] = res.results[c]["outT"].T[:NPC]
    return out


if __name__ == "__main__":
    rng = np.random.default_rng(0)
    demo = dict(
        node_feat=rng.standard_normal((N, D)).astype(np.float32),
        src=rng.integers(0, N, E).astype(np.int32),
        dst=rng.integers(0, N, E).astype(np.int32),
        edge_feat=rng.standard_normal((K, E, D)).astype(np.float32),
    )
    for nm, sh in (("Wq", (D, D)), ("bq", (D,)), ("gq", (D,)), ("beta_q", (D,)),
                   ("Wk", (D, D)), ("bk", (D,)), ("gk", (D,)), ("beta_k", (D,)),
                   ("Wv", (D, D)), ("bv", (D,))):
        demo[nm] = (rng.standard_normal(sh) * 0.05).astype(np.float32)
    demo["mlp_W1"] = (rng.standard_normal((3, K, D, D)) * 0.05).astype(np.float32)
    demo["mlp_b1"] = np.zeros((3, K, D), np.float32)
    demo["mlp_g"] = np.ones((3, K, D), np.float32)
    demo["mlp_beta"] = np.zeros((3, K, D), np.float32)
    demo["mlp_W2"] = (rng.standard_normal((3, K, D, D)) * 0.05).astype(np.float32)
    demo["mlp_b2"] = np.zeros((3, K, D), np.float32)
    demo["comb_W1"] = (rng.standard_normal((2 * D, 2 * D)) * 0.05).astype(np.float32)
    demo["comb_b1"] = np.zeros((2 * D,), np.float32)
    demo["comb_W2"] = (rng.standard_normal((2 * D, D)) * 0.05).astype(np.float32)
    out = kernel(**demo)
    print("out", out.shape, out.dtype, float(np.abs(out).mean()))



# revision 12
# speedup vs baseline: 1.4860x; 1.4860x over previous
"""CrAKNConvV2 GNN message-passing kernel for 8 trn2 NeuronCores (Bass/Tile).

Self-contained; shapes hardcoded for N=40000, E=200000, D=128, K=2.

Strategy:
  - Edges sharded across 8 cores by dst octile (5000 nodes/core), sorted by
    dst and packed into 512-edge macro-tiles whose dst window spans <= 128
    local nodes.  Segment-sum becomes a one-hot [e,u] matmul per macro plus a
    128-row indirect scatter into a per-core h table (no all-reduce needed).
  - Node stage (q/k/v) computed per-octile; q AllGathered (bf16) so each core
    can gather q[src] for arbitrary global src.
  - Edge MLPs in "D-on-partition" layout: tiles [do, e] with weights as
    stationary lhsT; LN mean folded into W1 on the host; variance via a
    replicated ones-matmul; rsqrt on ACT; mish via a (5,4) rational with
    custom DVE ops (bf16 interior) + fast reciprocal.
  - Engine balance: ACT (scalar) does squares/rsqrt and all PSUM->SBUF
    evictions; DVE does the mish rational + fused bias ops
    (scalar_tensor_tensor folds the LN bias in); GpSimd only gathers;
    final W2 matmuls emitted in e-layout (chunked lhsT) so no transposes
    are needed before the val multiply.
  - Comb MLP on the local h slice; output returned transposed per core and
    assembled on the host.
"""
import sys

sys.path.insert(0, "/opt/trn_rl_repo")

import numpy as np
import ml_dtypes

import concourse.bass as bass
import concourse.bacc as bacc
import concourse.tile as tile
import concourse.mybir as mybir
from concourse.bass_utils import run_bass_kernel_spmd
from concourse.masks import make_identity

bf16 = ml_dtypes.bfloat16
dt = mybir.dt
AF = mybir.ActivationFunctionType
ALU = mybir.AluOpType

N, E, D, K = 40000, 200000, 128, 2
NC = 8
NPC = N // NC            # 5000 nodes per core
V = 5120                 # padded local node-table rows (real: 0..4999)
SUB = 128
MAC = 512                # edges per macro tile (4 subtiles)
NCM = V // MAC           # node/comb macros per core (10)

# (5,4) rational fit of mish on [-7, 7], max abs err 2.2e-3:
#   mish(x) ~= S * x*((((x+a)x+b)x+c)x+d) / ((((x+q3)x+q2)x+q1)x+q0)
MISH_A = 19.166871033044686
MISH_B = 153.65177897609692
MISH_C = 677.6408201640626
MISH_D = 1431.337455343249
MISH_Q3 = -5.812366573366565
MISH_Q2 = 144.26844999380356
MISH_Q1 = -42.68836425004621
MISH_Q0 = 836.7746892765476
MISH_S = 0.35327729144764103

LN_EPS = 1e-5
RAD_EPS = 1e-8

_MISH_OPS = {}


def _register_mish_ops():
    """Register MISH_DEN / MISH_NUM custom DVE ops (monic quartic Horner with
    the constant term spilled to in1)."""
    if _MISH_OPS:
        return _MISH_OPS
    import concourse.dve_ops as dve_ops
    from concourse.dve_spec import Spec, Src0, Src1, C0, C1, C2, lower
    from concourse.dve_uop import DveOpSpec

    def _ref_den(in0, in1, s0, s1, imm2):
        x = in0.astype(np.float32)
        return ((((x + s0) * x + s1) * x + imm2) * x) + in1

    def _ref_num(in0, in1, s0, s1, imm2):
        x = in0.astype(np.float32)
        return (((((x + s0) * x + s1) * x + imm2) * x) + in1) * x

    body_den = ((((Src0 + C0) * Src0 + C1) * Src0 + C2) * Src0) + Src1
    body_num = (((((Src0 + C0) * Src0 + C1) * Src0 + C2) * Src0) + Src1) * Src0

    made = []
    for name, body, ref in (
        ("MISH_DEN_ANT", body_den, _ref_den),
        ("MISH_NUM_ANT", body_num, _ref_num),
    ):
        existing = [o for o in dve_ops.OPS if o.name == name]
        if existing:
            made.append(existing[0])
            continue
        row = max(dve_ops._SUB_OPCODE_FOR_NAME.values()) + 1
        assert row < 0x20
        dve_ops._SUB_OPCODE_FOR_NAME[name] = row
        spec = Spec(body=body, reference=ref)
        shas = {}
        for ver in ("v3", "v4"):
            try:
                shas[ver] = DveOpSpec(name=name, opcode=row,
                                      uops=lower(spec, ver=ver),
                                      rd1_en=True).sha(ver)
            except Exception:
                pass
        op = dve_ops.DveOp(name, spec, subdim=False, uops_sha=shas)
        dve_ops.OPS.append(op)
        dve_ops.CUSTOM_DVE_SPECS[name] = spec
        made.append(op)
    _MISH_OPS["den"] = made[0]
    _MISH_OPS["num"] = made[1]
    return _MISH_OPS


# ---------------------------------------------------------------- host prep
def _fold_ln_w(W, b):
    return W - W.mean(axis=1, keepdims=True), b - b.mean()


def _pack_core(deg):
    macros = []
    n = 0
    e0 = 0
    while n < NPC:
        first = n
        edges = 0
        while n < NPC and (n - first) < SUB and edges + deg[n] <= MAC:
            edges += int(deg[n])
            n += 1
        assert n > first, f"node {n} has degree {deg[n]} > {MAC}"
        macros.append((first, n - first, e0, edges))
        e0 += edges
    return macros


def _prep(inputs):
    nf = np.ascontiguousarray(np.asarray(inputs["node_feat"], np.float32))
    src = np.asarray(inputs["src"], np.int32)
    dst = np.asarray(inputs["dst"], np.int32)
    ef = np.asarray(inputs["edge_feat"], np.float32)

    Wq, bq = _fold_ln_w(np.asarray(inputs["Wq"], np.float32), np.asarray(inputs["bq"], np.float32))
    Wk, bk = _fold_ln_w(np.asarray(inputs["Wk"], np.float32), np.asarray(inputs["bk"], np.float32))
    Wv = np.asarray(inputs["Wv"], np.float32)
    bv = np.asarray(inputs["bv"], np.float32)
    gq, beq = np.asarray(inputs["gq"], np.float32), np.asarray(inputs["beta_q"], np.float32)
    gk, bek = np.asarray(inputs["gk"], np.float32), np.asarray(inputs["beta_k"], np.float32)

    mW1 = np.asarray(inputs["mlp_W1"], np.float32).copy()   # [3,K,D,D]
    mb1 = np.asarray(inputs["mlp_b1"], np.float32).copy()   # [3,K,D]
    mg = np.asarray(inputs["mlp_g"], np.float32)
    mbe = np.asarray(inputs["mlp_beta"], np.float32)
    mW2 = np.asarray(inputs["mlp_W2"], np.float32)
    mb2 = np.asarray(inputs["mlp_b2"], np.float32)
    for i in range(3):
        for k in range(K):
            mW1[i, k], mb1[i, k] = _fold_ln_w(mW1[i, k], mb1[i, k])

    cW1 = np.asarray(inputs["comb_W1"], np.float32)
    cb1 = np.asarray(inputs["comb_b1"], np.float32)
    cW2 = np.asarray(inputs["comb_W2"], np.float32)

    gen_node = not (np.all(gq == 1) and np.all(beq == 0)
                    and np.all(gk == 1) and np.all(bek == 0))
    gen_edge = not (np.all(mg == 1) and np.all(mbe == 0))
    gen_nbias = not (np.all(bq == 0) and np.all(bk == 0) and np.all(bv == 0))
    gen_b2w = not np.all(mb2[2] == 0)
    gen_cb1 = not np.all(cb1 == 0)

    order = np.argsort(dst, kind="stable")
    dsts = dst[order]
    core_bounds = np.searchsorted(dsts, np.arange(NC + 1) * NPC)

    per_core = []
    NMs = []
    for c in range(NC):
        eidx = order[core_bounds[c]:core_bounds[c + 1]]
        dl = dsts[core_bounds[c]:core_bounds[c + 1]] - c * NPC
        deg = np.bincount(dl, minlength=NPC)
        macros = _pack_core(deg)
        per_core.append((eidx, dl, macros))
        NMs.append(len(macros))
    NM = max(NMs)

    cores = []
    for c in range(NC):
        eidx, dl, macros = per_core[c]
        nmc = len(macros)
        efT = np.zeros((K, D, NM * MAC), bf16)
        srcq = np.zeros((NM, SUB, 4), np.int32)
        win = np.zeros((NM, SUB, 1), np.int32)
        scat = np.zeros((NM, SUB, 1), np.int32)
        pool = np.zeros((NM, SUB, 4, SUB), bf16)
        poolT = np.zeros((NM, SUB, 4, SUB), bf16)
        ar = np.arange(SUB)
        for m in range(NM):
            if m < nmc:
                first, span, e0, ne = macros[m]
            else:
                first, span, e0, ne = NPC, 0, len(eidx), 0
            win[m, :, 0] = np.minimum(first + ar, V - 1)
            scat[m, :, 0] = np.where(ar < span, first + ar,
                                     NPC + ((ar - span) % (V - NPC)))
            if ne == 0:
                continue
            ge = eidx[e0:e0 + ne]
            doff = dl[e0:e0 + ne] - first
            slots = np.arange(ne)
            cc, pp = slots // SUB, slots % SUB
            cols = m * MAC + slots
            for k in range(K):
                efT[k][:, cols] = ef[k][ge].T.astype(bf16)
            s_glob = src[ge]
            srcq[m][pp, cc] = (s_glob // NPC) * V + (s_glob % NPC)
            pool[m][pp, cc, doff] = 1
            poolT[m][doff, cc, pp] = 1
        nfT = np.zeros((D, V), bf16)
        nfT[:, :NPC] = nf[c * NPC:(c + 1) * NPC].T.astype(bf16)
        cores.append(dict(
            efT=efT, srcq=srcq, win=win, scat=scat,
            pool=pool.reshape(NM, SUB, 4 * SUB),
            poolT=poolT.reshape(NM, SUB, 4 * SUB), nfT=nfT))

    shared = dict(
        wq=Wq.astype(bf16), wk=Wk.astype(bf16), wv=Wv.astype(bf16),
        nbias=np.stack([bq, bk, bv])[None, :, :].astype(bf16),     # [1,3,D]
        ngam=np.stack([gq, gk])[None, :, :].astype(bf16),          # [1,2,D]
        nbet=np.stack([beq, bek])[None, :, :].astype(bf16),        # [1,2,D]
        mW1=mW1.transpose(2, 0, 1, 3).reshape(D, 3 * K * D).astype(bf16).copy(),
        mW2=mW2.transpose(2, 0, 1, 3).reshape(D, 3 * K * D).astype(bf16).copy(),
        mscal=np.stack([mb1, mb2, mg, mbe], axis=0)                # [4,3,K,D]
            .transpose(3, 0, 1, 2).reshape(D, 4 * 3 * K)
            .astype(np.float32).copy(),
        cW1=cW1.reshape(2, D, 2, D).transpose(1, 0, 2, 3)
            .reshape(D, 4 * D).astype(bf16).copy(),                # [di, (kc,mc,do)]
        cW2=cW2.reshape(2, D, D).transpose(1, 0, 2)
            .reshape(D, 2 * D).astype(bf16).copy(),                # [di, (mc,do)]
        cb1=cb1.reshape(2, D).T.astype(np.float32).copy(),         # [D, 2]
        b2wrep=np.broadcast_to(mb2[2][None, :, :], (SUB, K, D))
            .astype(bf16).copy(),                                  # [SUB, K, D]
    )
    flags = (gen_node, gen_edge, gen_nbias, gen_b2w, gen_cb1)
    return cores, shared, NM, flags


# ---------------------------------------------------------------- builder
def _build(NM, flags):
    gen_node, gen_edge, gen_nbias, gen_b2w, gen_cb1 = flags
    ops = _register_mish_ops()
    from concourse.dve_ops import RECIPROCAL_APPROX_FAST, RECIP_APPROX_FAST_CONSTS
    RC = RECIP_APPROX_FAST_CONSTS

    nc = bacc.Bacc("TRN2", target_bir_lowering=False, debug=False,
                   enable_asserts=False, num_devices=NC)

    t_nfT = nc.dram_tensor("nfT", [D, V], dt.bfloat16, kind="ExternalInput")
    t_wq = nc.dram_tensor("wq", [D, D], dt.bfloat16, kind="ExternalInput")
    t_wk = nc.dram_tensor("wk", [D, D], dt.bfloat16, kind="ExternalInput")
    t_wv = nc.dram_tensor("wv", [D, D], dt.bfloat16, kind="ExternalInput")
    t_nbias = nc.dram_tensor("nbias", [1, 3, D], dt.bfloat16, kind="ExternalInput")
    t_ngam = nc.dram_tensor("ngam", [1, 2, D], dt.bfloat16, kind="ExternalInput")
    t_nbet = nc.dram_tensor("nbet", [1, 2, D], dt.bfloat16, kind="ExternalInput")
    t_efT = nc.dram_tensor("efT", [K, D, NM * MAC], dt.bfloat16, kind="ExternalInput")
    t_srcq = nc.dram_tensor("srcq", [NM, SUB, 4], dt.int32, kind="ExternalInput")
    t_win = nc.dram_tensor("win", [NM, SUB, 1], dt.int32, kind="ExternalInput")
    t_scat = nc.dram_tensor("scat", [NM, SUB, 1], dt.int32, kind="ExternalInput")
    t_pool = nc.dram_tensor("pool", [NM, SUB, 4 * SUB], dt.bfloat16, kind="ExternalInput")
    t_poolT = nc.dram_tensor("poolT", [NM, SUB, 4 * SUB], dt.bfloat16, kind="ExternalInput")
    t_mW1 = nc.dram_tensor("mW1", [D, 3 * K * D], dt.bfloat16, kind="ExternalInput")
    t_mW2 = nc.dram_tensor("mW2", [D, 3 * K * D], dt.bfloat16, kind="ExternalInput")
    t_mscal = nc.dram_tensor("mscal", [D, 4 * 3 * K], dt.float32, kind="ExternalInput")
    t_cW1 = nc.dram_tensor("cW1", [D, 4 * D], dt.bfloat16, kind="ExternalInput")
    t_cW2 = nc.dram_tensor("cW2", [D, 2 * D], dt.bfloat16, kind="ExternalInput")
    t_cb1 = nc.dram_tensor("cb1", [D, 2], dt.float32, kind="ExternalInput")
    t_b2wrep = nc.dram_tensor("b2wrep", [SUB, K, D], dt.bfloat16, kind="ExternalInput")
    t_out = nc.dram_tensor("outT", [D, V], dt.float32, kind="ExternalOutput")

    with tile.TileContext(nc) as tc:
        with (
            tc.tile_pool(name="const", bufs=1) as cp,
            tc.tile_pool(name="sb", bufs=3) as sb,
            tc.tile_pool(name="mlp", bufs=2) as mp,
            tc.tile_pool(name="psU", bufs=2, space="PSUM") as psU,
            tc.tile_pool(name="psM", bufs=1, space="PSUM") as psM,
            tc.tile_pool(name="psP", bufs=1, space="PSUM") as psP,
            tc.tile_pool(name="psW", bufs=1, space="PSUM") as psW,
            tc.tile_pool(name="psS", bufs=1, space="PSUM") as psS,
            tc.tile_pool(name="dram", bufs=1, space="DRAM") as dram,
        ):
            # ---------------- constants
            ones = cp.tile([D, D], dt.bfloat16)
            nc.vector.memset(ones[:], 1.0)
            onesrow = cp.tile([1, D], dt.bfloat16)
            nc.vector.memset(onesrow[:], 1.0)
            ident = cp.tile([D, D], dt.bfloat16)
            make_identity(nc, ident[:])
            eps_ln = cp.tile([D, 1], dt.float32)
            nc.vector.memset(eps_ln[:], LN_EPS)
            eps_rad = cp.tile([D, 1], dt.float32)
            nc.vector.memset(eps_rad[:], RAD_EPS)
            mishd = cp.tile([D, 1], dt.float32)
            nc.vector.memset(mishd[:], MISH_D)
            mishq0 = cp.tile([D, 1], dt.float32)
            nc.vector.memset(mishq0[:], MISH_Q0)

            wq_t = cp.tile([D, D], dt.bfloat16); nc.sync.dma_start(wq_t[:], t_wq[:])
            wk_t = cp.tile([D, D], dt.bfloat16); nc.sync.dma_start(wk_t[:], t_wk[:])
            wv_t = cp.tile([D, D], dt.bfloat16); nc.sync.dma_start(wv_t[:], t_wv[:])
            nbias_t = cp.tile([1, 3, D], dt.bfloat16); nc.sync.dma_start(nbias_t[:], t_nbias[:])
            ngam_t = cp.tile([1, 2, D], dt.bfloat16); nc.sync.dma_start(ngam_t[:], t_ngam[:])
            nbet_t = cp.tile([1, 2, D], dt.bfloat16); nc.sync.dma_start(nbet_t[:], t_nbet[:])
            mW1_t = cp.tile([D, 3 * K * D], dt.bfloat16); nc.sync.dma_start(mW1_t[:], t_mW1[:])
            mW2_t = cp.tile([D, 3 * K * D], dt.bfloat16); nc.sync.dma_start(mW2_t[:], t_mW2[:])
            mscal_t = cp.tile([D, 4 * 3 * K], dt.float32); nc.sync.dma_start(mscal_t[:], t_mscal[:])
            cW1_t = cp.tile([D, 4 * D], dt.bfloat16); nc.sync.dma_start(cW1_t[:], t_cW1[:])
            cW2_t = cp.tile([D, 2 * D], dt.bfloat16); nc.sync.dma_start(cW2_t[:], t_cW2[:])
            cb1_t = cp.tile([D, 2], dt.float32); nc.sync.dma_start(cb1_t[:], t_cb1[:])
            b2wrep_t = cp.tile([SUB, K, D], dt.bfloat16)
            if gen_b2w:
                nc.sync.dma_start(b2wrep_t[:], t_b2wrep[:])

            def msc(which, i, k):
                col = which * 6 + i * K + k
                return mscal_t[:, col:col + 1]

            def w1s(i, k):
                b = (i * K + k) * D
                return mW1_t[:, b:b + D]

            def w2s(i, k):
                b = (i * K + k) * D
                return mW2_t[:, b:b + D]

            def cw1s(kc, mc):
                b = (kc * 2 + mc) * D
                return cW1_t[:, b:b + D]

            if gen_node:
                ngb = []
                for j in range(2):
                    gp = psP.tile([D, D], dt.float32, tag="pem", space="PSUM")
                    nc.tensor.matmul(gp[:], onesrow[:], ngam_t[:, j, :], start=True, stop=True)
                    gs = cp.tile([D, D], dt.bfloat16, tag=f"ngb{j}")
                    nc.vector.tensor_copy(gs[:], gp[:])
                    bp = psP.tile([D, D], dt.float32, tag="pem", space="PSUM")
                    nc.tensor.matmul(bp[:], onesrow[:], nbet_t[:, j, :], start=True, stop=True)
                    bs = cp.tile([D, D], dt.bfloat16, tag=f"nbb{j}")
                    nc.vector.tensor_copy(bs[:], bp[:])
                    ngb.append((gs, bs))

            q_local = dram.tile([V, D], dt.bfloat16)
            q_all = dram.tile([NC * V, D], dt.bfloat16, addr_space="Shared")
            kv_local = dram.tile([V, 2 * D], dt.bfloat16)
            h_local = dram.tile([V, 2 * D], dt.bfloat16)

            # ---------------- mish helper (bf16 interior, 1x custom ops)
            def mish_apply(x_ap, P, F):
                qd = mp.tile([P, F], dt.bfloat16, tag="mish_q")
                nc.vector._custom_dve(ops["den"], out=qd[:], in0=x_ap,
                                      in1=mishq0[:P].to_broadcast([P, F]),
                                      s0=MISH_Q3, s1=MISH_Q2, imm2=MISH_Q1)
                rr = mp.tile([P, F], dt.bfloat16, tag="mish_r")
                nc.vector._custom_dve(RECIPROCAL_APPROX_FAST, out=rr[:], in0=qd[:],
                                      s0=RC["s0"], s1=RC["s1"], imm2=RC["imm2"])
                pn = mp.tile([P, F], dt.bfloat16, tag="mish_p")
                nc.vector._custom_dve(ops["num"], out=pn[:], in0=x_ap,
                                      in1=mishd[:P].to_broadcast([P, F]),
                                      s0=MISH_A, s1=MISH_B, imm2=MISH_C)
                z = mp.tile([P, F], dt.bfloat16, tag="mish_z")
                nc.vector.scalar_tensor_tensor(z[:], pn[:], MISH_S, rr[:],
                                               op0=ALU.mult, op1=ALU.mult)
                return z

            # LN+mish for a PAIR of [D, MAC] psum pre-activations.
            # b1_a/b1_b: [D,1] bias column APs. Returns z tile [D, 2*MAC] bf16.
            def ln_mish_pair(u_a, u_b, b1_a, b1_b, gsc=None):
                x = mp.tile([D, 2, MAC], dt.bfloat16, tag="ex")
                for j, (u, b1) in enumerate(((u_a, b1_a), (u_b, b1_b))):
                    y2 = mp.tile([D, MAC], dt.bfloat16, tag="ey2")
                    nc.scalar.activation(y2[:], u[:], AF.Square, bias=b1)
                    mt = psM.tile([D, MAC], dt.float32, tag="mt", space="PSUM")
                    nc.tensor.matmul(mt[:], ones[:], y2[:], start=True, stop=True)
                    s = mp.tile([D, MAC], dt.bfloat16, tag="es")
                    nc.scalar.activation(s[:], mt[:], AF.Abs_reciprocal_sqrt,
                                         bias=eps_ln[:], scale=1.0 / D)
                    nc.vector.scalar_tensor_tensor(x[:, j, :], u[:], b1, s[:],
                                                   op0=ALU.add, op1=ALU.mult)
                if gsc is not None:
                    (ga, bea), (gb, beb) = gsc
                    xg = mp.tile([D, 2, MAC], dt.bfloat16, tag="exg")
                    nc.vector.tensor_scalar(xg[:, 0, :], x[:, 0, :], ga, bea,
                                            op0=ALU.mult, op1=ALU.add)
                    nc.vector.tensor_scalar(xg[:, 1, :], x[:, 1, :], gb, beb,
                                            op0=ALU.mult, op1=ALU.add)
                    x = xg
                return mish_apply(x[:].rearrange("p c f -> p (c f)"), D, 2 * MAC)

            # ================ node stage ================
            for nm in range(NCM):
                n0 = nm * MAC
                nfT_sl = sb.tile([D, MAC], dt.bfloat16, tag="nf")
                nc.sync.dma_start(nfT_sl[:], t_nfT[:, n0:n0 + MAC])
                # q and k chains paired; v separate (no LN/mish)
                xn = mp.tile([SUB, 2, 4, D], dt.bfloat16, tag="nx")
                for chain in range(2):      # 0=q 1=k
                    W_t = (wq_t, wk_t)[chain]
                    u = psU.tile([SUB, 4, D], dt.float32, tag="eu", space="PSUM")
                    ss4 = sb.tile([SUB, 4], dt.float32, tag="node_ss")
                    s4 = sb.tile([SUB, 4], dt.float32, tag="node_s4")
                    y2n = sb.tile([SUB, D], dt.bfloat16, tag="node_y2")
                    for c in range(4):
                        nc.tensor.matmul(u[:, c, :], nfT_sl[:, c * SUB:(c + 1) * SUB],
                                         W_t[:], start=True, stop=not gen_nbias)
                        if gen_nbias:
                            nc.tensor.matmul(u[:, c, :], onesrow[:], nbias_t[:, chain, :],
                                             start=False, stop=True)
                        nc.scalar.activation(y2n[:], u[:, c, :], AF.Square,
                                             accum_out=ss4[:, c:c + 1])
                    nc.scalar.activation(s4[:], ss4[:], AF.Abs_reciprocal_sqrt,
                                         bias=eps_ln[:SUB], scale=1.0 / D)
                    for c in range(4):
                        nc.vector.tensor_scalar(xn[:, chain, c, :], u[:, c, :],
                                                s4[:, c:c + 1], None, op0=ALU.mult)
                    if gen_node:
                        gs, bs = ngb[chain]
                        for c in range(4):
                            xgn = sb.tile([SUB, D], dt.bfloat16, tag="node_xg")
                            nc.vector.tensor_tensor(xgn[:], xn[:, chain, c, :],
                                                    gs[:], op=ALU.mult)
                            nc.vector.tensor_tensor(xn[:, chain, c, :], xgn[:],
                                                    bs[:], op=ALU.add)
                zn = mish_apply(xn[:].rearrange("p j c d -> p (j c d)"), SUB, 8 * D)
                znv = zn[:].rearrange("p (j c d) -> p j c d", j=2, c=4)
                nc.sync.dma_start(
                    q_local[n0:n0 + MAC, :].rearrange("(c p) d -> p c d", p=SUB),
                    znv[:, 0, :, :])
                nc.sync.dma_start(
                    kv_local[n0:n0 + MAC, 0:D].rearrange("(c p) d -> p c d", p=SUB),
                    znv[:, 1, :, :])
                # v chain
                uv = psU.tile([SUB, 4, D], dt.float32, tag="eu", space="PSUM")
                for c in range(4):
                    nc.tensor.matmul(uv[:, c, :], nfT_sl[:, c * SUB:(c + 1) * SUB],
                                     wv_t[:], start=True, stop=not gen_nbias)
                    if gen_nbias:
                        nc.tensor.matmul(uv[:, c, :], onesrow[:], nbias_t[:, 2, :],
                                         start=False, stop=True)
                vsb = sb.tile([SUB, 4, D], dt.bfloat16, tag="vsb")
                nc.scalar.activation(vsb[:].rearrange("p c d -> p (c d)"),
                                     uv[:].rearrange("p c d -> p (c d)"), AF.Copy)
                nc.sync.dma_start(
                    kv_local[n0:n0 + MAC, D:2 * D].rearrange("(c p) d -> p c d", p=SUB),
                    vsb[:])

            # ================ allgather q ================
            nc.gpsimd.collective_compute(
                "AllGather", ALU.bypass,
                replica_groups=[list(range(NC))],
                ins=[q_local.opt()], outs=[q_all.opt()])

            # ================ edge stage ================
            for m in range(NM):
                sl = slice(m * MAC, (m + 1) * MAC)
                efk = []
                for k in range(K):
                    t = sb.tile([D, MAC], dt.bfloat16, tag=f"ef{k}")
                    nc.sync.dma_start(t[:], t_efT[k][:, sl])
                    efk.append(t)
                pool_sb = sb.tile([SUB, 4 * SUB], dt.bfloat16, tag="pool")
                nc.sync.dma_start(pool_sb[:], t_pool[m])
                poolT_sb = sb.tile([SUB, 4 * SUB], dt.bfloat16, tag="poolT")
                nc.sync.dma_start(poolT_sb[:], t_poolT[m])
                srcq_sb = sb.tile([SUB, 4], dt.int32, tag="srcq")
                nc.sync.dma_start(srcq_sb[:], t_srcq[m])
                win_sb = sb.tile([SUB, 1], dt.int32, tag="win")
                nc.sync.dma_start(win_sb[:], t_win[m])
                scat_sb = sb.tile([SUB, 1], dt.int32, tag="scat")
                nc.sync.dma_start(scat_sb[:], t_scat[m])

                qg = sb.tile([SUB, 4, D], dt.bfloat16, tag="qg")
                for c in range(4):
                    nc.gpsimd.indirect_dma_start(
                        out=qg[:, c, :], out_offset=None, in_=q_all[:],
                        in_offset=bass.IndirectOffsetOnAxis(
                            ap=srcq_sb[:, c:c + 1], axis=0))
                kvw = sb.tile([SUB, 2 * D], dt.bfloat16, tag="kvw")
                nc.gpsimd.indirect_dma_start(
                    out=kvw[:], out_offset=None, in_=kv_local[:],
                    in_offset=bass.IndirectOffsetOnAxis(ap=win_sb[:], axis=0))

                # expand window k/v to edges: kvexp = poolT^T @ kvw (per chunk)
                ke = sb.tile([SUB, 4, D], dt.bfloat16, tag="ke")
                ve = sb.tile([SUB, 4, D], dt.bfloat16, tag="ve")
                for half in range(2):
                    kvexp = psS.tile([SUB, 2, 2 * D], dt.float32, tag="kvexp",
                                     space="PSUM")
                    for cc in range(2):
                        c = half * 2 + cc
                        nc.tensor.matmul(kvexp[:, cc, :],
                                         poolT_sb[:, c * SUB:(c + 1) * SUB],
                                         kvw[:], start=True, stop=True)
                    nc.scalar.activation(ke[:, half * 2:half * 2 + 2, :],
                                         kvexp[:, :, 0:D], AF.Copy)
                    nc.scalar.activation(ve[:, half * 2:half * 2 + 2, :],
                                         kvexp[:, :, D:2 * D], AF.Copy)

                rel = sb.tile([SUB, 4, D], dt.bfloat16, tag="rel")
                nc.vector.tensor_tensor(rel[:], qg[:], ke[:], op=ALU.subtract)
                rel2 = sb.tile([SUB, 4, D], dt.bfloat16, tag="rel2")
                nc.vector.tensor_tensor(rel2[:], rel[:], rel[:], op=ALU.mult)
                rad = sb.tile([SUB, 4], dt.float32, tag="rad")
                nc.vector.tensor_reduce(rad[:], rel2[:], axis=mybir.AxisListType.X,
                                        op=ALU.add)
                invr = sb.tile([SUB, 4], dt.float32, tag="invr")
                nc.scalar.activation(invr[:], rad[:], AF.Abs_reciprocal_sqrt,
                                     bias=eps_rad[:SUB], scale=1.0)
                reln = sb.tile([SUB, 4, D], dt.bfloat16, tag="reln")
                for c in range(4):
                    nc.vector.tensor_scalar(reln[:, c, :], rel[:, c, :],
                                            invr[:, c:c + 1], None, op0=ALU.mult)
                relT_ps = psS.tile([D, 4, SUB], dt.bfloat16, tag="kvexp", space="PSUM")
                for c in range(4):
                    nc.tensor.transpose(relT_ps[:, c, :], reln[:, c, :], ident[:])
                relT = sb.tile([D, 4 * SUB], dt.bfloat16, tag="relTs")
                nc.scalar.activation(relT[:], relT_ps[:].rearrange("p c e -> p (c e)"),
                                     AF.Copy)

                # ---- m/b chains per key -> rcomb
                rcombs = []
                for k in range(K):
                    um = psU.tile([D, MAC], dt.float32, tag="eu", space="PSUM")
                    nc.tensor.matmul(um[:], w1s(0, k), efk[k][:], start=True, stop=True)
                    ub = psU.tile([D, MAC], dt.float32, tag="eu", space="PSUM")
                    nc.tensor.matmul(ub[:], w1s(1, k), efk[k][:], start=True, stop=True)
                    gsc = None
                    if gen_edge:
                        gsc = ((msc(2, 0, k), msc(3, 0, k)),
                               (msc(2, 1, k), msc(3, 1, k)))
                    zmb = ln_mish_pair(um, ub, msc(0, 0, k), msc(0, 1, k), gsc)
                    pem = psP.tile([D, MAC], dt.float32, tag="pem", space="PSUM")
                    nc.tensor.matmul(pem[:], w2s(0, k), zmb[:, 0:MAC], start=True, stop=True)
                    peb = psP.tile([D, MAC], dt.float32, tag="peb", space="PSUM")
                    nc.tensor.matmul(peb[:], w2s(1, k), zmb[:, MAC:2 * MAC], start=True, stop=True)
                    rhalf = mp.tile([D, MAC], dt.bfloat16, tag="rh")
                    nc.vector.scalar_tensor_tensor(rhalf[:], pem[:], msc(1, 0, k),
                                                   relT[:], op0=ALU.add, op1=ALU.mult)
                    rcomb = mp.tile([D, MAC], dt.bfloat16, tag=f"rc{k}")
                    nc.vector.scalar_tensor_tensor(rcomb[:], peb[:], msc(1, 1, k),
                                                   rhalf[:], op0=ALU.add, op1=ALU.add)
                    rcombs.append(rcomb)

                # ---- w chains, both keys paired
                uw0 = psU.tile([D, MAC], dt.float32, tag="eu", space="PSUM")
                nc.tensor.matmul(uw0[:], w1s(2, 0), rcombs[0][:], start=True, stop=True)
                uw1 = psU.tile([D, MAC], dt.float32, tag="eu", space="PSUM")
                nc.tensor.matmul(uw1[:], w1s(2, 1), rcombs[1][:], start=True, stop=True)
                gsc = None
                if gen_edge:
                    gsc = ((msc(2, 2, 0), msc(3, 2, 0)),
                           (msc(2, 2, 1), msc(3, 2, 1)))
                zw = ln_mish_pair(uw0, uw1, msc(0, 2, 0), msc(0, 2, 1), gsc)

                # ---- final W2 in e-layout (chunked lhsT), val, segment-sum
                valt = sb.tile([SUB, 4, K, D], dt.bfloat16, tag="valt")
                for k in range(K):
                    wu = psW.tile([SUB, 4, D], dt.float32, tag="wu", space="PSUM")
                    for c in range(4):
                        nc.tensor.matmul(wu[:, c, :],
                                         zw[:, k * MAC + c * SUB:k * MAC + (c + 1) * SUB],
                                         w2s(2, k), start=True, stop=True)
                    if gen_b2w:
                        # fallback: b2w replicated across partitions (host input)
                        wub = sb.tile([SUB, 4, D], dt.bfloat16, tag="wub")
                        for c in range(4):
                            nc.vector.tensor_tensor(wub[:, c, :], wu[:, c, :],
                                                    b2wrep_t[:, k, :], op=ALU.add)
                        nc.vector.tensor_tensor(valt[:, :, k, :], wub[:], ve[:],
                                                op=ALU.mult)
                    else:
                        nc.vector.tensor_tensor(valt[:, :, k, :], wu[:], ve[:],
                                                op=ALU.mult)

                H = psS.tile([SUB, K * D], dt.float32, tag="H", space="PSUM")
                for c in range(4):
                    nc.tensor.matmul(H[:], pool_sb[:, c * SUB:(c + 1) * SUB],
                                     valt[:, c, :, :].rearrange("p k d -> p (k d)"),
                                     start=(c == 0), stop=(c == 3))
                Hs = sb.tile([SUB, 2 * D], dt.bfloat16, tag="Hs")
                nc.scalar.activation(Hs[:], H[:], AF.Copy)
                nc.gpsimd.indirect_dma_start(
                    out=h_local[:], out_offset=bass.IndirectOffsetOnAxis(
                        ap=scat_sb[:], axis=0),
                    in_=Hs[:], in_offset=None)

            # ================ comb stage ================
            for cm in range(NCM):
                n0 = cm * MAC
                hT = []
                for j in range(2):
                    t = sb.tile([D, MAC], dt.bfloat16, tag=f"hT{j}")
                    nc.sync.dma_start_transpose(
                        t[:], h_local[n0:n0 + MAC, j * D:(j + 1) * D])
                    hT.append(t)
                zc = []
                for mc in range(2):
                    cu = psU.tile([D, MAC], dt.float32, tag="eu", space="PSUM")
                    nc.tensor.matmul(cu[:], cw1s(0, mc), hT[0][:], start=True, stop=False)
                    nc.tensor.matmul(cu[:], cw1s(1, mc), hT[1][:], start=False, stop=True)
                    if gen_cb1:
                        xc = mp.tile([D, MAC], dt.bfloat16, tag="xc")
                        nc.vector.tensor_scalar(xc[:], cu[:], cb1_t[:, mc:mc + 1],
                                                None, op0=ALU.add)
                        zc.append(mish_apply(xc[:], D, MAC))
                    else:
                        zc.append(mish_apply(cu[:], D, MAC))
                ou = psP.tile([D, MAC], dt.float32, tag="pem", space="PSUM")
                nc.tensor.matmul(ou[:], cW2_t[:, 0:D], zc[0][:], start=True, stop=False)
                nc.tensor.matmul(ou[:], cW2_t[:, D:2 * D], zc[1][:], start=False, stop=True)
                osb = sb.tile([D, MAC], dt.float32, tag="osb")
                nc.scalar.activation(osb[:], ou[:], AF.Copy)
                nc.sync.dma_start(t_out[:, n0:n0 + MAC], osb[:])

    nc.compile()
    return nc


_CACHE = {}


def kernel(**inputs) -> np.ndarray:
    cores, shared, NM, flags = _prep(inputs)
    key = (NM, flags)
    if key not in _CACHE:
        _CACHE[key] = _build(NM, flags)
    nc = _CACHE[key]
    in_maps = []
    for c in range(NC):
        m = dict(shared)
        m.update(cores[c])
        in_maps.append(m)
    res = run_bass_kernel_spmd(nc, in_maps, core_ids=list(range(NC)))
    out = np.empty((N, D), np.float32)
    for c in range(NC):
        out[c * NPC:(c + 1) * NPC] = res.results[c]["outT"].T[:NPC]
    return out


if __name__ == "__main__":
    rng = np.random.default_rng(0)
    demo = dict(
        node_feat=rng.standard_normal((N, D)).astype(np.float32),
        src=rng.integers(0, N, E).astype(np.int32),
        dst=rng.integers(0, N, E).astype(np.int32),
        edge_feat=rng.standard_normal((K, E, D)).astype(np.float32),
    )
    for nm, sh in (("Wq", (D, D)), ("bq", (D,)), ("gq", (D,)), ("beta_q", (D,)),
                   ("Wk", (D, D)), ("bk", (D,)), ("gk", (D,)), ("beta_k", (D,)),
                   ("Wv", (D, D)), ("bv", (D,))):
        demo[nm] = (rng.standard_normal(sh) * 0.05).astype(np.float32)
    demo["mlp_W1"] = (rng.standard_normal((3, K, D, D)) * 0.05).astype(np.float32)
    demo["mlp_b1"] = np.zeros((3, K, D), np.float32)
    demo["mlp_g"] = np.ones((3, K, D), np.float32)
    demo["mlp_beta"] = np.zeros((3, K, D), np.float32)
    demo["mlp_W2"] = (rng.standard_normal((3, K, D, D)) * 0.05).astype(np.float32)
    demo["mlp_b2"] = np.zeros((3, K, D), np.float32)
    demo["comb_W1"] = (rng.standard_normal((2 * D, 2 * D)) * 0.05).astype(np.float32)
    demo["comb_b1"] = np.zeros((2 * D,), np.float32)
    demo["comb_W2"] = (rng.standard_normal((2 * D, D)) * 0.05).astype(np.float32)
    out = kernel(**demo)
    print("out", out.shape, out.dtype, float(np.abs(out).mean()))
